# revision 1
# baseline (speedup 1.0000x reference)
"""nn_AggEncoder kernel: PointNet-style GNN on 8 NeuronCores, data-parallel over batch.

Self-contained: accepts FULL unsharded inputs, shards batch across the 8
devices, runs the forward, gathers the FULL output.
"""

import functools

import jax
import jax.numpy as jnp
import numpy as np

B, N, K, H = 8, 4096, 16, 64
EPS_LN = 1e-5
EPS_STD = 1e-5


def _ln(x, g, b):
    m = jnp.mean(x, -1, keepdims=True)
    v = jnp.mean((x - m) ** 2, -1, keepdims=True)
    return (x - m) * jax.lax.rsqrt(v + EPS_LN) * g + b


def _res_block(x, W1, b1, g1, be1, W2, b2, g2, be2):
    h = jax.nn.relu(_ln(x @ W1 + b1, g1, be1))
    return jax.nn.relu(x + _ln(h @ W2 + b2, g2, be2))


def _forward(xyz, x, W_emb, b_emb, g_emb, be_emb, alpha, beta,
             W_pre, b_pre, g_pre, be_pre,
             W1a, b1a, g1a, be1a, W2a, b2a, g2a, be2a,
             W_pos, b_pos, g_pos, be_pos,
             W1b, b1b, g1b, be1b, W2b, b2b, g2b, be2b,
             W_fin, b_fin, g_fin, be_fin):
    pts = jax.nn.relu(_ln(x @ W_emb + b_emb, g_emb, be_emb))
    sq = jnp.sum(xyz ** 2, -1)
    sqd = sq[:, :, None] + sq[:, None, :] - 2.0 * jnp.einsum('bnd,bmd->bnm', xyz, xyz)
    _, idx = jax.lax.top_k(-sqd, K)
    grouped = jax.vmap(lambda p, i: p[i])(pts, idx)
    diff = grouped - pts[:, :, None, :]
    std = jnp.std(diff.reshape(diff.shape[0], -1), axis=-1, ddof=1)[:, None, None, None]
    grouped = diff / (std + EPS_STD) * alpha + beta
    newp = jnp.concatenate(
        [grouped, jnp.broadcast_to(pts[:, :, None, :], grouped.shape)], axis=-1)
    y = _ln(newp @ W_pre + b_pre, g_pre, be_pre)
    y = _res_block(y, W1a, b1a, g1a, be1a, W2a, b2a, g2a, be2a)
    y = jnp.max(y, axis=2)
    y = _ln(y @ W_pos + b_pos, g_pos, be_pos)
    y = _res_block(y, W1b, b1b, g1b, be1b, W2b, b2b, g2b, be2b)
    y = jnp.max(y, axis=1)
    return _ln(y @ W_fin + b_fin, g_fin, be_fin)


_ARG_NAMES = [
    "xyz", "x", "W_emb", "b_emb", "g_emb", "be_emb", "alpha", "beta",
    "W_pre", "b_pre", "g_pre", "be_pre",
    "W1a", "b1a", "g1a", "be1a", "W2a", "b2a", "g2a", "be2a",
    "W_pos", "b_pos", "g_pos", "be_pos",
    "W1b", "b1b", "g1b", "be1b", "W2b", "b2b", "g2b", "be2b",
    "W_fin", "b_fin", "g_fin", "be_fin",
]

_compiled = None


def _get_compiled():
    global _compiled
    if _compiled is not None:
        return _compiled
    devices = jax.devices()[:8]
    mesh = jax.sharding.Mesh(np.array(devices), ("b",))
    P = jax.sharding.PartitionSpec
    batch_sharding = jax.sharding.NamedSharding(mesh, P("b"))
    rep = jax.sharding.NamedSharding(mesh, P())
    in_shardings = tuple(
        batch_sharding if nm in ("xyz", "x") else rep for nm in _ARG_NAMES
    )
    fn = jax.jit(
        _forward,
        in_shardings=in_shardings,
        out_shardings=batch_sharding,
    )
    _compiled = (fn, mesh)
    return _compiled


def kernel(**inputs: np.ndarray) -> np.ndarray:
    fn, _ = _get_compiled()
    args = [jnp.asarray(inputs[nm]) for nm in _ARG_NAMES]
    out = fn(*args)
    return np.asarray(jax.device_get(out)).astype(np.float32)
